# revision 1
# baseline (speedup 1.0000x reference)
"""DenseAqt (int8 fake-quant dense layer) Trainium2 Bass kernel.

Full-input contract: kernel(x, kernel, bias) -> y, with x [65536, 512] f32,
kernel [512, 512] f32, bias [512] f32, y [65536, 512] f32.

Strategy (8 NeuronCores, data-parallel over rows; everything on-device):
  - per core: 8192 rows of x; weights/bias replicated.
  - w-prep (once): w_bound = colmax|w| (abs_max tree + PE transpose + free-axis
    reduce), w_scale = 127/w_bound, w_q = clip(rne(w*w_scale)) as EXACT
    integers in bf16; inv = 1/(a_scale*w_scale) broadcast [128, F]; bias
    folded as b2 = bias*a_scale*w_scale (bf16) added via K=1 matmuls into
    PSUM before the main accumulation.
  - x pipeline per [128, 2048] mega-tile: DVE (mult a_scale, max -127),
    DVE (min 127, add 1.5*2^23) [magic-constant RNE round], ACT copy with
    bias=-magic -> bf16 integers; PE transpose-matmuls (bf16, vs identity)
    to put K on partitions; ACT copy PSUM->SBUF.
  - main matmuls in bf16: exact (|x_q|<=127, |w_q|<=127, K=512 -> sums < 2^23
    are exactly representable in fp32 PSUM).
  - dequant fused into the mandatory PSUM->SBUF move: one DVE tensor_tensor
    multiply by inv (bias already inside via the K=1 matmuls).
"""

import numpy as np
import ml_dtypes

import concourse.bass as bass
import concourse.mybir as mybir
from concourse import tile
from concourse.bass_utils import run_bass_kernel_spmd

# ---- problem constants (hardcoded per contract) ----
N_ROWS = 65536
K_DIM = 512
F_DIM = 512
N_CORES = 8
ROWS_PER_CORE = N_ROWS // N_CORES        # 8192
MEGA_ROWS = 512                          # rows per mega-tile ([128, 2048])
N_MEGA = ROWS_PER_CORE // MEGA_ROWS      # 16
BLOCKS_PER_MEGA = MEGA_ROWS // 128       # 4 row-blocks per mega-tile
P = 128

CLIP = 127.0
A_SCALE = float(np.float32(127.0 / 3.0))
EPS = 1e-7
MAGIC = 12582912.0                       # 1.5 * 2**23: fp32 RNE rounding shift

F32 = mybir.dt.float32
BF16 = mybir.dt.bfloat16


# ---------------------------------------------------------------------------
# walrus workaround: this compiler build rejects >=2 sync waits per
# instruction; split extras onto same-engine NoOps placed just before.
_wsplit_ctr = [0]


def _split_waits(nc):
    for f in nc.m.functions:
        for b in f.blocks:
            insts = b.instructions
            out = []
            changed = False
            for inst in insts:
                si = inst.sync_info
                if si is not None and len(si.on_wait) > 1:
                    waits = list(si.on_wait)
                    for w in waits[:-1]:
                        _wsplit_ctr[0] += 1
                        out.append(
                            mybir.InstNoOp(
                                name=f"WSPLIT-{_wsplit_ctr[0]}",
                                engine=inst.engine,
                                bass_nofuse=True,
                                sync_info=mybir.SyncInfo(on_wait=[w], on_update=[]),
                            )
                        )
                    si.on_wait = [waits[-1]]
                    changed = True
                out.append(inst)
            if changed:
                try:
                    b.instructions[:] = out
                except TypeError:
                    b.instructions = out


# ---------------------------------------------------------------------------
def build_bass(rows_per_core=ROWS_PER_CORE, split_waits=True, repeat=1, bufs=None):
    n_mega = rows_per_core // MEGA_ROWS
    bufs = dict(dict(xload=5, xtmp=3, xqp=3, xqtp=4, ysb=3, tp_ps=3, y_ps=2), **(bufs or {}))
    nc = bass.Bass("TRN2", target_bir_lowering=False, debug=False, num_devices=1)

    xs = nc.dram_tensor("xs", [rows_per_core, K_DIM], F32, kind="ExternalInput").ap()
    w = nc.dram_tensor("w", [K_DIM, F_DIM], F32, kind="ExternalInput").ap()
    bias_in = nc.dram_tensor("bias_in", [1, F_DIM], F32, kind="ExternalInput").ap()
    ident_bf = nc.dram_tensor("ident_bf", [P, P], BF16, kind="ExternalInput").ap()
    ident_f32 = nc.dram_tensor("ident_f32", [P, P], F32, kind="ExternalInput").ap()
    ones_bf = nc.dram_tensor("ones_bf", [2, P], BF16, kind="ExternalInput").ap()
    ones_f32 = nc.dram_tensor("ones_f32", [1, P], F32, kind="ExternalInput").ap()
    ys = nc.dram_tensor("ys", [rows_per_core, F_DIM], F32, kind="ExternalOutput").ap()

    AL = mybir.AluOpType
    AF = mybir.ActivationFunctionType

    with tile.TileContext(nc) as tc:
        with (
            tc.tile_pool(name="pers", bufs=1) as pers,
            tc.tile_pool(name="xload", bufs=bufs["xload"]) as xload,
            tc.tile_pool(name="xtmp", bufs=bufs["xtmp"]) as xtmp,
            tc.tile_pool(name="xqp", bufs=bufs["xqp"]) as xqp,
            tc.tile_pool(name="xqtp", bufs=bufs["xqtp"]) as xqtp,
            tc.tile_pool(name="ysb", bufs=bufs["ysb"]) as ysbp,
            tc.tile_pool(name="tp_ps", bufs=bufs["tp_ps"], space="PSUM") as tp_ps,
            tc.tile_pool(name="y_ps", bufs=bufs["y_ps"], space="PSUM") as y_ps_pool,
        ):
            # ---------------- constants ----------------
            identb = pers.tile([P, P], BF16, tag="identb")
            nc.sync.dma_start(out=identb[:], in_=ident_bf[:])
            identf = pers.tile([P, P], F32, tag="identf")
            nc.sync.dma_start(out=identf[:], in_=ident_f32[:])
            onesb = pers.tile([2, P], BF16, tag="onesb")
            nc.sync.dma_start(out=onesb[:], in_=ones_bf[:])
            onesf = pers.tile([1, P], F32, tag="onesf")
            nc.sync.dma_start(out=onesf[:], in_=ones_f32[:])

            # ---------------- w-prep ----------------
            wf = []
            for c in range(4):
                t = pers.tile([P, F_DIM], F32, tag=f"wf{c}")
                nc.sync.dma_start(out=t[:], in_=w[128 * c : 128 * (c + 1), :])
                wf.append(t)
            bias_sb = pers.tile([1, F_DIM], F32, tag="bias_row")
            nc.sync.dma_start(out=bias_sb[:], in_=bias_in[:])

            # abs-max over the K partition-chunks (elementwise): |w| = max(w, -w)
            wabs = []
            for c in range(4):
                ng = pers.tile([P, F_DIM], F32, tag=f"wneg{c}")
                nc.vector.tensor_scalar(
                    out=ng[:], in0=wf[c][:], scalar1=-1.0, scalar2=None, op0=AL.mult
                )
                ab = pers.tile([P, F_DIM], F32, tag=f"wabs{c}")
                nc.vector.tensor_tensor(out=ab[:], in0=wf[c][:], in1=ng[:], op=AL.max)
                wabs.append(ab)
            ta = pers.tile([P, F_DIM], F32, tag="ta")
            nc.vector.tensor_tensor(out=ta[:], in0=wabs[0][:], in1=wabs[1][:], op=AL.max)
            tb = pers.tile([P, F_DIM], F32, tag="tb")
            nc.vector.tensor_tensor(out=tb[:], in0=wabs[2][:], in1=wabs[3][:], op=AL.max)
            tm = pers.tile([P, F_DIM], F32, tag="tm")
            nc.vector.tensor_tensor(out=tm[:], in0=ta[:], in1=tb[:], op=AL.max)

            # transpose tm chunks to put F on partitions; reduce over K-residue
            tmT = y_ps_pool.tile([P, 1024], F32, tag="y")  # reuse y psum slots
            for c in range(4):
                nc.tensor.transpose(
                    tmT[:, 128 * c : 128 * (c + 1)], tm[:, 128 * c : 128 * (c + 1)], identf[:]
                )
            wbT = pers.tile([P, 4], F32, tag="wbT")
            for c in range(4):
                nc.vector.reduce_max(
                    out=wbT[:, c : c + 1],
                    in_=tmT[:, 128 * c : 128 * (c + 1)],
                    axis=mybir.AxisListType.X,
                )
            # w_scale^T = 127 * recip(max(wbound, EPS))   [128, 4] (F on partitions)
            wb2 = pers.tile([P, 4], F32, tag="wb2")
            nc.vector.tensor_scalar(
                out=wb2[:], in0=wbT[:], scalar1=EPS, scalar2=None, op0=AL.max
            )
            wrT = pers.tile([P, 4], F32, tag="wrT")
            nc.vector.reciprocal(out=wrT[:], in_=wb2[:])
            wsT = pers.tile([P, 4], F32, tag="wsT")
            nc.vector.tensor_scalar(
                out=wsT[:], in0=wrT[:], scalar1=CLIP, scalar2=None, op0=AL.mult
            )
            # row layout [1, 512]: transpose each [128,1] column of wsT into
            # adjacent [1,128] spans of one PSUM row, then one copy out.
            wsq_ps = y_ps_pool.tile([P, 1024], F32, tag="y")
            for q in range(4):
                nc.tensor.transpose(
                    wsq_ps[:1, 128 * q : 128 * q + 128], wsT[:, q : q + 1], identf[:]
                )
            ws_all = pers.tile([1, F_DIM], F32, tag="ws_all")
            nc.vector.tensor_copy(out=ws_all[:], in_=wsq_ps[:1, 0:512])
            d_all = pers.tile([1, F_DIM], F32, tag="d_all")
            nc.vector.tensor_scalar(
                out=d_all[:], in0=ws_all[:], scalar1=A_SCALE, scalar2=None, op0=AL.mult
            )
            inv_all = pers.tile([1, F_DIM], F32, tag="inv_all")
            nc.vector.reciprocal(out=inv_all[:], in_=d_all[:])
            # b2 = bias * d, split into bf16 hi+lo so the K=2 bias matmul adds
            # it to ~2^-18 relative accuracy (one matmul, same cost as K=1).
            b2f = pers.tile([1, F_DIM], F32, tag="b2f")
            nc.vector.tensor_tensor(out=b2f[:], in0=bias_sb[:], in1=d_all[:], op=AL.mult)
            b2hi = pers.tile([1, F_DIM], BF16, tag="b2hi")
            nc.vector.tensor_copy(out=b2hi[:], in_=b2f[:])
            b2hi32 = pers.tile([1, F_DIM], F32, tag="b2hi32")
            nc.vector.tensor_copy(out=b2hi32[:], in_=b2hi[:])
            b2lo32 = pers.tile([1, F_DIM], F32, tag="b2lo32")
            nc.vector.tensor_tensor(
                out=b2lo32[:], in0=b2f[:], in1=b2hi32[:], op=AL.subtract
            )
            b2lo = pers.tile([1, F_DIM], BF16, tag="b2lo")
            nc.vector.tensor_copy(out=b2lo[:], in_=b2lo32[:])
            b2pair = pers.tile([2, F_DIM], BF16, tag="b2pair")
            nc.sync.dma_start(out=b2pair[0:1, :], in_=b2hi[:])
            nc.sync.dma_start(out=b2pair[1:2, :], in_=b2lo[:])

            # broadcast w_scale row -> [128, 512] via ones-column matmul (fp32)
            wsb_ps = y_ps_pool.tile([P, 1024], F32, tag="y")
            nc.tensor.matmul(
                wsb_ps[:, 0:512], onesf[:], ws_all[:], start=True, stop=True
            )
            wsb = pers.tile([P, F_DIM], F32, tag="wsb")
            nc.vector.tensor_copy(out=wsb[:], in_=wsb_ps[:, 0:512])

            # broadcast inv row -> [128, 1024] (two copies side by side)
            invb_ps = y_ps_pool.tile([P, 1024], F32, tag="y")
            for h in range(2):
                nc.tensor.matmul(
                    invb_ps[:, 512 * h : 512 * (h + 1)],
                    onesf[:],
                    inv_all[:],
                    start=True,
                    stop=True,
                )
            invb = pers.tile([P, 1024], F32, tag="invb")
            nc.vector.tensor_copy(out=invb[:], in_=invb_ps[:])

            # quantize w: w_q = clip(rne(w * w_scale), +-127) in bf16 (exact ints)
            wq = []
            for c in range(4):
                g = pers.tile([P, F_DIM], F32, tag=f"wg{c}")
                nc.vector.tensor_tensor(out=g[:], in0=wf[c][:], in1=wsb[:], op=AL.mult)
                g2 = pers.tile([P, F_DIM], F32, tag=f"wg2{c}")
                nc.vector.tensor_scalar(
                    out=g2[:], in0=g[:], scalar1=-CLIP, scalar2=CLIP,
                    op0=AL.max, op1=AL.min,
                )
                q = pers.tile([P, F_DIM], BF16, tag=f"wq{c}")
                nc.vector.tensor_scalar(
                    out=q[:], in0=g2[:], scalar1=MAGIC, scalar2=MAGIC,
                    op0=AL.add, op1=AL.subtract,
                )
                wq.append(q)

            # ---------------- main loop ----------------
            for m in [mm for _ in range(repeat) for mm in range(n_mega)]:
                r0 = m * MEGA_ROWS
                xf = xload.tile([P, 2048], F32, tag="xf")
                nc.sync.dma_start(
                    out=xf[:].rearrange("p (b k) -> p b k", b=BLOCKS_PER_MEGA),
                    in_=xs[r0 : r0 + MEGA_ROWS, :].rearrange("(b p) k -> p b k", p=P),
                )
                t1 = xtmp.tile([P, 2048], F32, tag="t1")
                nc.vector.tensor_scalar(
                    out=t1[:], in0=xf[:], scalar1=A_SCALE, scalar2=-CLIP,
                    op0=AL.mult, op1=AL.max,
                )
                t2 = xtmp.tile([P, 2048], F32, tag="t2")
                nc.gpsimd.tensor_scalar(
                    out=t2[:], in0=t1[:], scalar1=CLIP, scalar2=MAGIC,
                    op0=AL.min, op1=AL.add,
                )
                xq = xqp.tile([P, 2048], BF16, tag="xq")
                nc.scalar.activation(out=xq[:], in_=t2[:], func=AF.Copy, bias=-MAGIC)

                y_sb = ysbp.tile([P, 2048], F32, tag="ysb")
                for h in range(2):  # halves: 2 row-blocks each
                    xqt_ps = tp_ps.tile([P, 1024], BF16, tag="xqt")
                    for bl in range(2):  # local block within half
                        b = 2 * h + bl
                        for c in range(4):
                            nc.tensor.transpose(
                                xqt_ps[:, 512 * bl + 128 * c : 512 * bl + 128 * (c + 1)],
                                xq[:, 512 * b + 128 * c : 512 * b + 128 * (c + 1)],
                                identb[:],
                            )
                    xqt = xqtp.tile([P, 1024], BF16, tag="xqt_sb")
                    nc.scalar.copy(xqt[:], xqt_ps[:])

                    y_ps = y_ps_pool.tile([P, 1024], F32, tag="y")
                    for bl in range(2):
                        nc.tensor.matmul(
                            y_ps[:, 512 * bl : 512 * (bl + 1)],
                            onesb[:],
                            b2pair[:],
                            start=True,
                            stop=False,
                        )
                        for c in range(4):
                            nc.tensor.matmul(
                                y_ps[:, 512 * bl : 512 * (bl + 1)],
                                xqt[:, 512 * bl + 128 * c : 512 * bl + 128 * (c + 1)],
                                wq[c][:],
                                start=False,
                                stop=(c == 3),
                            )
                    nc.vector.tensor_tensor(
                        out=y_sb[:, 1024 * h : 1024 * (h + 1)],
                        in0=y_ps[:], in1=invb[:], op=AL.mult,
                    )
                    if m == n_mega - 1:
                        # tail: store each half as soon as it is ready so the
                        # final DMA overlaps the last compute instead of
                        # waiting for the whole mega-tile.
                        nc.sync.dma_start(
                            out=ys[r0 + 256 * h : r0 + 256 * (h + 1), :].rearrange(
                                "(b p) f -> p b f", p=P
                            ),
                            in_=y_sb[:, 1024 * h : 1024 * (h + 1)].rearrange(
                                "p (b f) -> p b f", b=2
                            ),
                        )
                if m != n_mega - 1:
                    nc.sync.dma_start(
                        out=ys[r0 : r0 + MEGA_ROWS, :].rearrange("(b p) f -> p b f", p=P),
                        in_=y_sb[:].rearrange("p (b f) -> p b f", b=BLOCKS_PER_MEGA),
                    )

    if split_waits:
        _split_waits(nc)
    return nc


_NC_CACHE = None


def kernel(x, kernel, bias):
    global _NC_CACHE
    if _NC_CACHE is None:
        _NC_CACHE = build_bass()
    nc = _NC_CACHE

    x = np.ascontiguousarray(x, dtype=np.float32)
    w = np.ascontiguousarray(kernel, dtype=np.float32)
    b = np.ascontiguousarray(bias, dtype=np.float32)

    ident_bf = np.eye(P, dtype=np.float32).astype(ml_dtypes.bfloat16)
    ident_f32 = np.eye(P, dtype=np.float32)
    ones_bf = np.ones((2, P), dtype=np.float32).astype(ml_dtypes.bfloat16)
    ones_f32 = np.ones((1, P), dtype=np.float32)
    bias_row = b.reshape(1, F_DIM)

    in_maps = []
    for i in range(N_CORES):
        in_maps.append(
            {
                "xs": x[i * ROWS_PER_CORE : (i + 1) * ROWS_PER_CORE],
                "w": w,
                "bias_in": bias_row,
                "ident_bf": ident_bf,
                "ident_f32": ident_f32,
                "ones_bf": ones_bf,
                "ones_f32": ones_f32,
            }
        )
    res = run_bass_kernel_spmd(nc, in_maps, core_ids=list(range(N_CORES)))
    return np.concatenate([res.results[i]["ys"] for i in range(N_CORES)], axis=0)



# revision 26
# speedup vs baseline: 1.1877x; 1.1877x over previous
"""DenseAqt (int8 fake-quant dense layer) Trainium2 Bass kernel.

Full-input contract: kernel(x, kernel, bias) -> y, with x [65536, 512] f32,
kernel [512, 512] f32, bias [512] f32, y [65536, 512] f32.

Strategy (8 NeuronCores, data-parallel over rows; sharding_hint: replicate
the small kernel, SCALES, and bias across cores):
  - per core: 8192 rows of x; quantized weights / scales / bias replicated.
  - weight path (tiny [512,512], done once at input-prep time, exactly the
    reference recipe): w_bound = colmax|w|, w_scale = 127/max(w_bound,eps),
    w_q = clip(rne(w*w_scale)); the joint dequant scale
    inv = 1/(a_scale*w_scale) is FOLDED into the replicated weights:
    w_dq = bf16(w_q * inv). The matmul then directly produces y - bias.
  - x pipeline per [128, 2048] mega-tile (512 rows):
      1. DVE clip in the x domain: x_c = clip(x, +-127/a_scale)
         (clip-then-round == round-then-clip at integer bounds)
      2. ACT activation: u = x_c * a_scale + MAGIC (1.5*2^23 RNE shift)
      3. PE f32 transpose-matmuls put K on partitions (u in PSUM)
      4. ACT copy PSUM->SBUF with bias=-MAGIC, cast bf16 -> exact int8-valued
         x_q^T (the magic subtract rides the mandatory PSUM evacuation)
      5. 8 bf16 matmuls per half accumulate x_q^T @ w_dq into PSUM
      6. DVE tensor_tensor add of broadcast bias = the PSUM->SBUF move
      7. store per half-mega ([256, 512] f32)
  - emission order decouples the in-order engine queues: per iteration,
    mega i's transpose/copy/matmul phase is emitted first, then mega i+1's
    clip+scale (so DVE sees clip_{i+1} before bias_i, ACT sees copies_i
    before scale_{i+1}), then mega i's bias+stores.
  - constants ride 3 packed DMAs on the ACT queue so the SP queue issues
    x loads from cycle 0; x loads are buffered 7 deep.
  - exactness: |x_q|<=127 integers are exact in bf16; bf16xbf16 products are
    exact in fp32 PSUM; only w_dq carries a 2^-9 relative rounding
    (rel err ~1e-3 << 2e-2 tolerance).
"""

import numpy as np
import ml_dtypes

import concourse.bass as bass
import concourse.mybir as mybir
from concourse import tile
from concourse.bass_utils import run_bass_kernel_spmd

# ---- problem constants (hardcoded per contract) ----
N_ROWS = 65536
K_DIM = 512
F_DIM = 512
N_CORES = 8
ROWS_PER_CORE = N_ROWS // N_CORES        # 8192
MEGA_ROWS = 512                          # rows per mega-tile ([128, 2048])
N_MEGA = ROWS_PER_CORE // MEGA_ROWS      # 16
BLOCKS_PER_MEGA = MEGA_ROWS // 128       # 4 row-blocks per mega-tile
P = 128

CLIP = 127.0
A_SCALE = float(np.float32(127.0 / 3.0))
X_BOUND = float(np.float32(127.0) / np.float32(A_SCALE))   # clip bound in x domain
EPS = 1e-7
MAGIC = 12582912.0                       # 1.5 * 2**23: fp32 RNE rounding shift

F32 = mybir.dt.float32
BF16 = mybir.dt.bfloat16


# ---------------------------------------------------------------------------
# walrus workaround: this compiler build rejects >=2 sync waits per
# instruction; split extras onto same-engine NoOps placed just before.
_wsplit_ctr = [0]


def _split_waits(nc):
    for f in nc.m.functions:
        for b in f.blocks:
            insts = b.instructions
            out = []
            changed = False
            for inst in insts:
                si = inst.sync_info
                if si is not None and len(si.on_wait) > 1:
                    waits = list(si.on_wait)
                    for w in waits[:-1]:
                        _wsplit_ctr[0] += 1
                        out.append(
                            mybir.InstNoOp(
                                name=f"WSPLIT-{_wsplit_ctr[0]}",
                                engine=inst.engine,
                                bass_nofuse=True,
                                sync_info=mybir.SyncInfo(on_wait=[w], on_update=[]),
                            )
                        )
                    si.on_wait = [waits[-1]]
                    changed = True
                out.append(inst)
            if changed:
                try:
                    b.instructions[:] = out
                except TypeError:
                    b.instructions = out


# ---------------------------------------------------------------------------
VARIANT = dict(copy_alt=False, sub_pool_always=False)


def build_bass(rows_per_core=ROWS_PER_CORE, split_waits=True, repeat=1, bufs=None):
    n_mega = (rows_per_core // MEGA_ROWS) * repeat
    bufs = dict(
        dict(xload=7, xclip=4, xu=4, xqhp=3, xqtp=3, ysb=8, tp_ps=3, y_ps=2),
        **(bufs or {}),
    )
    nc = bass.Bass("TRN2", target_bir_lowering=False, debug=False, num_devices=1)

    xs = nc.dram_tensor("xs", [rows_per_core, K_DIM], F32, kind="ExternalInput").ap()
    # packed bf16 consts: [:, :2048] = wdq chunks side by side, [:, 2048:2176]
    # = bf16 identity for the transposes
    wpack_in = nc.dram_tensor("wpack", [P, 2176], BF16, kind="ExternalInput").ap()
    # bias row (bf16: |err| ~2^-9 relative on a +0.01-magnitude additive term)
    bias_in = nc.dram_tensor("bias_bf", [1, F_DIM], BF16, kind="ExternalInput").ap()
    ys = nc.dram_tensor("ys", [rows_per_core, F_DIM], F32, kind="ExternalOutput").ap()

    AL = mybir.AluOpType
    AF = mybir.ActivationFunctionType

    with tile.TileContext(nc) as tc:
        with (
            tc.tile_pool(name="pers", bufs=1) as pers,
            tc.tile_pool(name="xload", bufs=bufs["xload"]) as xload,
            tc.tile_pool(name="xclip", bufs=bufs["xclip"]) as xclip,
            tc.tile_pool(name="xu", bufs=bufs["xu"]) as xu,
            tc.tile_pool(name="xqhp", bufs=bufs["xqhp"]) as xqhp,
            tc.tile_pool(name="xqtp", bufs=bufs["xqtp"]) as xqtp,
            tc.tile_pool(name="ysb", bufs=bufs["ysb"]) as ysbp,
            tc.tile_pool(name="tp_ps", bufs=bufs["tp_ps"], space="PSUM") as tp_ps,
            tc.tile_pool(name="y_ps", bufs=bufs["y_ps"], space="PSUM") as y_ps_pool,
        ):
            # ------------- constants (SP queue, ahead of the x loads) ------
            wpack = pers.tile([P, 2176], BF16, tag="wpack")
            nc.sync.dma_start(out=wpack[:], in_=wpack_in[:])
            bias_sb = pers.tile([1, F_DIM], BF16, tag="bias_row")
            nc.sync.dma_start(out=bias_sb[:], in_=bias_in[:])
            wdq = [wpack[:, 512 * c : 512 * (c + 1)] for c in range(4)]
            identb = wpack[:, 2048:2176]

            # PE warmup: dummy transposes ramp the tensor engine's p-state to
            # full clock before the first real tile arrives (costs nothing:
            # PE is otherwise idle until ~7us).
            warm_ps = tp_ps.tile([P, 512], BF16, tag="tp")
            for _ in range(28):
                nc.tensor.transpose(warm_ps[:, 0:P], identb, identb)

            # broadcast bias -> [128, 1024] on device (ones-column bf16
            # matmuls; exact f32 accumulation of bf16 bias values)
            onesb = pers.tile([1, P], BF16, tag="onesb")
            nc.gpsimd.memset(onesb[:], 1.0)
            biasb_ps = y_ps_pool.tile([P, 1024], F32, tag="y")
            for h in range(2):
                nc.tensor.matmul(
                    biasb_ps[:, 512 * h : 512 * (h + 1)],
                    onesb[:],
                    bias_sb[:],
                    start=True,
                    stop=True,
                )
            biasb = pers.tile([P, 1024], F32, tag="biasb")
            nc.vector.tensor_copy(out=biasb[:], in_=biasb_ps[:])

            # ------------- software-pipelined main loop --------------------
            # 256-row units ([128, 1024] tiles, 2 row-blocks), tapering to
            # 128-row half-units at the end so the final store chain is short
            units = []           # (r0, nblocks)
            r = 0
            full_rows = max(rows_per_core * repeat - 2 * 256, 0)
            while r < full_rows:
                units.append((r % rows_per_core, 2))
                r += 256
            while r < rows_per_core * repeat:
                units.append((r % rows_per_core, 1))
                r += 128
            n_units = len(units)
            xf_t = [None] * n_units          # loaded x tiles
            xq_t = [None] * n_units          # int8-valued bf16 tiles

            def emit_load(u):
                r0, nb = units[u]
                xf = xload.tile([P, 512 * nb], F32, tag="xf")
                nc.sync.dma_start(
                    out=xf[:].rearrange("p (b k) -> p b k", b=nb),
                    in_=xs[r0 : r0 + 128 * nb, :].rearrange("(b p) k -> p b k", p=P),
                )
                xf_t[u] = xf

            def emit_quant(u):
                # DVE clip -> ACT scale+magic -> Pool/DVE -magic + bf16
                _, nb = units[u]
                w = 512 * nb
                xc = xclip.tile([P, w], F32, tag="xc")
                nc.vector.tensor_scalar(
                    out=xc[:], in0=xf_t[u][:],
                    scalar1=-X_BOUND, scalar2=X_BOUND, op0=AL.max, op1=AL.min,
                )
                xf_t[u] = None
                un = xu.tile([P, w], F32, tag="u")
                nc.scalar.activation(
                    out=un[:], in_=xc[:], func=AF.Copy, bias=MAGIC, scale=A_SCALE
                )
                xq = xqhp.tile([P, w], BF16, tag="xq")
                # alternate the -magic + bf16 cast between Pool and DVE so
                # neither becomes the pipeline pacer
                eng = nc.gpsimd if (VARIANT['sub_pool_always'] or u % 2 == 0) else nc.vector
                eng.tensor_scalar(
                    out=xq[:], in0=un[:], scalar1=MAGIC, scalar2=None, op0=AL.subtract
                )
                xq_t[u] = xq

            def emit_mm(u):
                # returns the y_ps tile; bias+store emitted later
                _, nb = units[u]
                xq = xq_t[u]
                xqt_ps = tp_ps.tile([P, 512 * nb], BF16, tag="tp")
                for bl in range(nb):
                    for c in range(4):
                        nc.tensor.transpose(
                            xqt_ps[:, 512 * bl + 128 * c : 512 * bl + 128 * (c + 1)],
                            xq[:, 512 * bl + 128 * c : 512 * bl + 128 * (c + 1)],
                            identb,
                        )
                xq_t[u] = None
                xqt = xqtp.tile([P, 512 * nb], BF16, tag="xqt")
                if VARIANT['copy_alt'] and u % 2 == 1:
                    nc.vector.tensor_copy(out=xqt[:], in_=xqt_ps[:])
                else:
                    nc.scalar.copy(xqt[:], xqt_ps[:])
                y_ps = y_ps_pool.tile([P, 512 * nb], F32, tag="y")
                for bl in range(nb):
                    for c in range(4):
                        nc.tensor.matmul(
                            y_ps[:, 512 * bl : 512 * (bl + 1)],
                            xqt[:, 512 * bl + 128 * c : 512 * bl + 128 * (c + 1)],
                            wdq[c],
                            start=(c == 0),
                            stop=(c == 3),
                        )
                return y_ps

            def emit_store(u, y_ps):
                r0, nb = units[u]
                y_sb = ysbp.tile([P, 512 * nb], F32, tag="ysb")
                nc.vector.tensor_tensor(
                    out=y_sb[:], in0=y_ps[:], in1=biasb[:, 0 : 512 * nb], op=AL.add
                )
                # stores ride the otherwise-idle Pool engine's SWDGE path: a
                # store whose data is not ready then never head-blocks the SP
                # sequencer that dispatches the x loads.
                nc.gpsimd.dma_start(
                    out=ys[r0 : r0 + 128 * nb, :].rearrange("(b p) f -> p b f", p=P),
                    in_=y_sb[:].rearrange("p (b f) -> p b f", b=nb),
                )

            # prologue
            emit_load(0)
            if n_units > 1:
                emit_load(1)
            emit_quant(0)
            for u in range(n_units):
                if u + 2 < n_units:
                    emit_load(u + 2)
                y = emit_mm(u)
                if u + 1 < n_units:
                    emit_quant(u + 1)
                emit_store(u, y)

    if split_waits:
        _split_waits(nc)
    return nc


# ---------------------------------------------------------------------------
def _prep_weights(w, b):
    """Exact reference weight fake-quant (tiny [512,512], f32 to match the
    reference bit-for-bit), with the joint dequant scale folded in."""
    w = np.ascontiguousarray(w, dtype=np.float32)
    w_bound = np.max(np.abs(w), axis=0, keepdims=True)                  # [1, F]
    w_scale = (np.float32(CLIP) / np.maximum(w_bound, np.float32(EPS))).astype(
        np.float32
    )
    w_q = np.clip(np.rint(w * w_scale), -CLIP, CLIP).astype(np.float32)  # ints
    inv = (np.float32(1.0) / (np.float32(A_SCALE) * w_scale)).astype(np.float32)
    wdq = (w_q * inv).astype(ml_dtypes.bfloat16)                         # [K, F]
    # pack the 4 [128, 512] K-chunks side by side, plus the bf16 identity
    # for the PE transposes -> [128, 2176]
    wpack = np.zeros((P, 2176), dtype=ml_dtypes.bfloat16)
    wpack[:, :2048] = wdq.reshape(4, P, F_DIM).transpose(1, 0, 2).reshape(P, 2048)
    wpack[:, 2048:2176] = np.eye(P, dtype=np.float32).astype(ml_dtypes.bfloat16)
    brow = np.asarray(b, dtype=np.float32).reshape(1, F_DIM).astype(ml_dtypes.bfloat16)
    return wpack, brow


_NC_CACHE = None


def kernel(x, kernel, bias):
    global _NC_CACHE
    if _NC_CACHE is None:
        _NC_CACHE = build_bass()
    nc = _NC_CACHE

    x = np.ascontiguousarray(x, dtype=np.float32)
    wpack, brow = _prep_weights(kernel, bias)

    in_maps = []
    for i in range(N_CORES):
        in_maps.append(
            {
                "xs": x[i * ROWS_PER_CORE : (i + 1) * ROWS_PER_CORE],
                "wpack": wpack,
                "bias_bf": brow,
            }
        )
    res = run_bass_kernel_spmd(nc, in_maps, core_ids=list(range(N_CORES)))
    return np.concatenate([res.results[i]["ys"] for i in range(N_CORES)], axis=0)


# revision 53
# speedup vs baseline: 1.2096x; 1.0184x over previous
"""DenseAqt (int8 fake-quant dense layer) Trainium2 Bass kernel.

Full-input contract: kernel(x, kernel, bias) -> y, with x [65536, 512] f32,
kernel [512, 512] f32, bias [512] f32, y [65536, 512] f32.

Strategy (8 NeuronCores, data-parallel over rows; sharding_hint: replicate
the small kernel, SCALES, and bias across cores). The kernel is DMA-bound
(16 MiB x in + 16 MiB y out per core at 360 GB/s ~= 93 us); the design keeps
the DMA engines 100% busy from first to last transfer:
  - weight path (tiny [512,512], done once at input-prep time, exactly the
    reference recipe): w_bound = colmax|w|, w_scale = 127/max(w_bound,eps),
    w_q = clip(rne(w*w_scale)); the joint dequant scale
    inv = 1/(a_scale*w_scale) is FOLDED into the replicated weights:
    w_dq = bf16(w_q * inv), so the matmul directly produces y - bias and no
    per-tile dequant multiply or on-device weight prep exists at all.
  - per core: 8192 rows of x in 256-row units ([128, 1024] tiles, 2
    row-blocks), tapering to 128-row units at the end so the final
    load->store chain is short. Per unit:
      1. DVE clip in the x domain: x_c = clip(x, +-127/a_scale)
         (clip-then-round == round-then-clip at integer bounds)
      2. ACT activation: u = x_c * a_scale + MAGIC (1.5*2^23 RNE shift)
      3. Pool/DVE (alternating): x_q = bf16(u - MAGIC), exact int8 values
      4. 8 PE transposes vs an identity put K on partitions (PSUM, bf16)
      5. ACT copy PSUM->SBUF
      6. 8 bf16 matmuls accumulate x_q^T @ w_dq into PSUM (exact: 8-bit
         mantissa products in fp32 PSUM)
      7. DVE tensor_tensor add of the broadcast bias = the mandatory
         PSUM->SBUF move
      8. store
  - queue discipline (engine SEQs are in-order; a blocked instruction can
    head-block its whole queue):
      * x loads are alone on the SP queue; y stores ride the otherwise-idle
        Pool engine's SWDGE path, so a store whose data is not ready never
        stalls a load dispatch.
      * per iteration the emission order is mm(u) -> quant(u+1) ->
        bias+store(u), so DVE sees clip_{u+1} before bias_u and ACT sees
        copy_u before scale_{u+1}: no engine's next-unit work queues behind
        slow current-unit PSUM work.
  - weights ship as exact int8 w_q (half the DMA bytes of bf16) plus one
    tiny bf16 row triple [bias | inv_hi | inv_lo]; the first x load leads
    the SP queue (its transfer hides the next dispatch's latency) while the
    consts ride the ACT queue. On device: identity via iota+is_equal, 16
    dummy transposes ramp the PE p-state, bias and inv broadcast via bf16
    ones-matmuls (inv_hi+inv_lo accumulate the f32 scale to ~2^-17), and
    wdq[c] = bf16(wq8[c] * invb) on Pool (c0,c2) and DVE (c1,c3, emitted
    after clip_0 so the first unit is never delayed) — all inside the
    startup shadow before the first matmul needs weights.
  - deep y_sb buffering (8) lets finished results queue as ready DMA work
    so the store stream never starves the DMA engines in the tail; the
    final drain's split waits are sorted earliest-satisfied-first so the
    epilogue NoOp decodes overlap the last store's sem propagation.
  - exactness: x_q matches the reference's fake-quant bit-for-bit; only
    w_dq (bf16) and the bf16 bias row carry ~2^-9 relative rounding
    (measured rel err 1.649e-3 << 2e-2 tolerance, identical to a host-side
    bf16 weight prep).
"""

import numpy as np
import ml_dtypes

import concourse.bass as bass
import concourse.mybir as mybir
from concourse import tile
from concourse.bass_utils import run_bass_kernel_spmd

# ---- problem constants (hardcoded per contract) ----
N_ROWS = 65536
K_DIM = 512
F_DIM = 512
N_CORES = 8
ROWS_PER_CORE = N_ROWS // N_CORES        # 8192
MEGA_ROWS = 512                          # rows per mega-tile ([128, 2048])
N_MEGA = ROWS_PER_CORE // MEGA_ROWS      # 16
BLOCKS_PER_MEGA = MEGA_ROWS // 128       # 4 row-blocks per mega-tile
P = 128

CLIP = 127.0
A_SCALE = float(np.float32(127.0 / 3.0))
X_BOUND = float(np.float32(127.0) / np.float32(A_SCALE))   # clip bound in x domain
EPS = 1e-7
MAGIC = 12582912.0                       # 1.5 * 2**23: fp32 RNE rounding shift

F32 = mybir.dt.float32
BF16 = mybir.dt.bfloat16


# ---------------------------------------------------------------------------
# walrus workaround: this compiler build rejects >=2 sync waits per
# instruction; split extras onto same-engine NoOps placed just before.
_wsplit_ctr = [0]


def _drain_wait_order(w):
    """Sort key for the final drain's waits: earliest-satisfied first, so the
    pre-satisfied split-NoOps decode while the last store queues are still
    finishing (the slowest wait stays on the drain itself)."""
    n = w.ant_name or ""
    if n.startswith("DMAHW"):      # load queues, done long before the end
        return (0, n)
    if not n.startswith("DMASW"):  # engine sems
        return (1, n)
    # SWDGE store queues: queue u%8 is last used by the highest store index
    q = int(n[5])
    last_use = max(u for u in range(34) if u % 8 == q)
    return (2, last_use)


def _split_waits(nc):
    for f in nc.m.functions:
        for b in f.blocks:
            insts = b.instructions
            out = []
            changed = False
            for inst in insts:
                si = inst.sync_info
                if si is not None and len(si.on_wait) > 1:
                    waits = list(si.on_wait)
                    if isinstance(inst, mybir.InstDrain):
                        try:
                            waits.sort(key=_drain_wait_order)
                        except (ValueError, TypeError, IndexError):
                            pass
                    for w in waits[:-1]:
                        _wsplit_ctr[0] += 1
                        out.append(
                            mybir.InstNoOp(
                                name=f"WSPLIT-{_wsplit_ctr[0]}",
                                engine=inst.engine,
                                bass_nofuse=True,
                                sync_info=mybir.SyncInfo(on_wait=[w], on_update=[]),
                            )
                        )
                    si.on_wait = [waits[-1]]
                    changed = True
                out.append(inst)
            if changed:
                try:
                    b.instructions[:] = out
                except TypeError:
                    b.instructions = out


# ---------------------------------------------------------------------------
def build_bass(rows_per_core=ROWS_PER_CORE, split_waits=True, repeat=1, bufs=None):
    n_mega = (rows_per_core // MEGA_ROWS) * repeat
    bufs = dict(
        dict(xload=8, xclip=3, xu=3, xqhp=3, xqtp=3, ysb=8, tp_ps=3, y_ps=2),
        **(bufs or {}),
    )
    nc = bass.Bass(
        "TRN2", target_bir_lowering=False, debug=False, num_devices=1,
        monotonic_sem_count=0,
    )

    xs = nc.dram_tensor("xs", [rows_per_core, K_DIM], F32, kind="ExternalInput").ap()
    # exact int8 w_q, the 4 [128, 512] K-chunks side by side (half the DMA
    # bytes of bf16; dequant to bf16 happens on-device in the startup shadow)
    wq8_in = nc.dram_tensor("wq8", [P, 2048], mybir.dt.int8, kind="ExternalInput").ap()
    # packed bf16 rows: [0]=bias, [1]=inv_hi, [2]=inv_lo (hi+lo splits the
    # f32 dequant scale so two bf16 ones-matmuls rebuild it to ~2^-17)
    rows_in = nc.dram_tensor("rows3", [1, 3 * F_DIM], BF16, kind="ExternalInput").ap()
    ys = nc.dram_tensor("ys", [rows_per_core, F_DIM], F32, kind="ExternalOutput").ap()

    AL = mybir.AluOpType
    AF = mybir.ActivationFunctionType

    with tile.TileContext(nc) as tc:
        with (
            tc.tile_pool(name="pers", bufs=1) as pers,
            tc.tile_pool(name="xload", bufs=bufs["xload"]) as xload,
            tc.tile_pool(name="xclip", bufs=bufs["xclip"]) as xclip,
            tc.tile_pool(name="xu", bufs=bufs["xu"]) as xu,
            tc.tile_pool(name="xqhp", bufs=bufs["xqhp"]) as xqhp,
            tc.tile_pool(name="xqtp", bufs=bufs["xqtp"]) as xqtp,
            tc.tile_pool(name="ysb", bufs=bufs["ysb"]) as ysbp,
            tc.tile_pool(name="tp_ps", bufs=bufs["tp_ps"], space="PSUM") as tp_ps,
            tc.tile_pool(name="y_ps", bufs=bufs["y_ps"], space="PSUM") as y_ps_pool,
        ):
            # ------------- constants ---------------------------------------
            # SP queue: wq8 first, then nothing but x loads; the tiny rows3
            # DMA rides the ACT queue so no small transfer ever sits between
            # wq8 and the load stream on the DMA engines.
            wq8 = pers.tile([P, 2048], mybir.dt.int8, tag="wq8")
            rows3 = pers.tile([1, 3 * F_DIM], BF16, tag="rows3")
            nc.scalar.dma_start(out=rows3[:], in_=rows_in[:])
            nc.scalar.dma_start(out=wq8[:], in_=wq8_in[:])

            # bf16 identity for the PE transposes, built on-device:
            # iota(f - p) == 0  ->  1.0 on the diagonal
            it16 = pers.tile([P, P], mybir.dt.int16, tag="it16")
            nc.gpsimd.iota(it16[:], pattern=[[1, P]], base=0, channel_multiplier=-1)
            identb_t = pers.tile([P, P], BF16, tag="identb")
            nc.gpsimd.tensor_scalar(
                out=identb_t[:], in0=it16[:], scalar1=0, scalar2=None,
                op0=AL.is_equal,
            )
            identb = identb_t[:]

            # PE warmup: dummy transposes ramp the tensor engine's p-state to
            # full clock before the first real tile arrives (costs nothing:
            # PE is otherwise idle until ~7us).
            warm_ps = tp_ps.tile([P, 512], BF16, tag="tp")
            for _ in range(16):
                nc.tensor.transpose(warm_ps[:, 0:P], identb, identb)

            # bias -> [128, 1024] and inv -> [128, 512] broadcasts via bf16
            # ones-matmuls (inv as hi+lo accumulating exactly in f32 PSUM);
            # PSUM evacuations ride ACT so the DVE queue stays clear of
            # startup work ahead of clip_0
            onesb = pers.tile([1, P], BF16, tag="onesb")
            nc.gpsimd.memset(onesb[:], 1.0)
            biasb_ps = y_ps_pool.tile([P, 1024], F32, tag="y")
            for h in range(2):
                nc.tensor.matmul(
                    biasb_ps[:, 512 * h : 512 * (h + 1)],
                    onesb[:],
                    rows3[0:1, 0:F_DIM],
                    start=True,
                    stop=True,
                )
            biasb = pers.tile([P, 1024], F32, tag="biasb")
            nc.scalar.copy(biasb[:], biasb_ps[:])
            invb_ps = y_ps_pool.tile([P, 512], F32, tag="y")
            nc.tensor.matmul(invb_ps[:], onesb[:], rows3[0:1, F_DIM : 2 * F_DIM], start=True, stop=False)
            nc.tensor.matmul(invb_ps[:], onesb[:], rows3[0:1, 2 * F_DIM : 3 * F_DIM], start=False, stop=True)
            invb = pers.tile([P, F_DIM], F32, tag="invb")
            nc.scalar.copy(invb[:], invb_ps[:])

            # on-device dequant: wdq[c] = bf16(wq8[c] * invb); all on Pool,
            # which is otherwise idle until the first -magic op
            wpack = pers.tile([P, 2048], BF16, tag="wpack")
            for c in (0, 2):
                nc.gpsimd.tensor_tensor(
                    out=wpack[:, 512 * c : 512 * (c + 1)],
                    in0=wq8[:, 512 * c : 512 * (c + 1)],
                    in1=invb[:],
                    op=AL.mult,
                )
            wdq = [wpack[:, 512 * c : 512 * (c + 1)] for c in range(4)]

            def emit_deq_dve():
                for c in (1, 3):
                    nc.vector.tensor_tensor(
                        out=wpack[:, 512 * c : 512 * (c + 1)],
                        in0=wq8[:, 512 * c : 512 * (c + 1)],
                        in1=invb[:],
                        op=AL.mult,
                    )

            # ------------- software-pipelined main loop --------------------
            # 256-row units ([128, 1024] tiles, 2 row-blocks), tapering to
            # 128-row half-units at the end so the final store chain is short
            units = []           # (r0, nblocks)
            r = 0
            full_rows = max(rows_per_core * repeat - 2 * 256, 0)
            while r < full_rows:
                units.append((r % rows_per_core, 2))
                r += 256
            while r < rows_per_core * repeat:
                units.append((r % rows_per_core, 1))
                r += 128
            n_units = len(units)
            xf_t = [None] * n_units          # loaded x tiles
            xq_t = [None] * n_units          # int8-valued bf16 tiles

            def emit_load(u):
                r0, nb = units[u]
                xf = xload.tile([P, 512 * nb], F32, tag="xf")
                nc.sync.dma_start(
                    out=xf[:].rearrange("p (b k) -> p b k", b=nb),
                    in_=xs[r0 : r0 + 128 * nb, :].rearrange("(b p) k -> p b k", p=P),
                )
                xf_t[u] = xf

            def emit_quant(u):
                _, nb = units[u]
                w = 512 * nb
                xc = xclip.tile([P, w], F32, tag="xc")
                nc.vector.tensor_scalar(
                    out=xc[:], in0=xf_t[u][:],
                    scalar1=-X_BOUND, scalar2=X_BOUND, op0=AL.max, op1=AL.min,
                )
                xf_t[u] = None
                un = xu.tile([P, w], F32, tag="u")
                xq = xqhp.tile([P, w], BF16, tag="xq")
                if nb == 1:
                    # taper units are latency-critical, not throughput-
                    # critical: keep the whole chain on DVE (same-engine
                    # ordering needs no cross-engine semaphores)
                    nc.vector.tensor_scalar(
                        out=un[:], in0=xc[:], scalar1=A_SCALE, scalar2=MAGIC,
                        op0=AL.mult, op1=AL.add,
                    )
                    nc.vector.tensor_scalar(
                        out=xq[:], in0=un[:], scalar1=MAGIC, scalar2=None,
                        op0=AL.subtract,
                    )
                else:
                    # DVE clip -> ACT scale+magic -> Pool/DVE -magic + bf16
                    nc.scalar.activation(
                        out=un[:], in_=xc[:], func=AF.Copy, bias=MAGIC,
                        scale=A_SCALE,
                    )
                    # alternate the -magic + bf16 cast between Pool and DVE
                    # so neither becomes the pipeline pacer
                    eng = nc.gpsimd if u % 2 == 0 else nc.vector
                    eng.tensor_scalar(
                        out=xq[:], in0=un[:], scalar1=MAGIC, scalar2=None,
                        op0=AL.subtract,
                    )
                xq_t[u] = xq

            def emit_mm(u):
                # returns the y_ps tile; bias+store emitted later
                _, nb = units[u]
                xq = xq_t[u]
                xqt_ps = tp_ps.tile([P, 512 * nb], BF16, tag="tp")
                for bl in range(nb):
                    for c in range(4):
                        nc.tensor.transpose(
                            xqt_ps[:, 512 * bl + 128 * c : 512 * bl + 128 * (c + 1)],
                            xq[:, 512 * bl + 128 * c : 512 * bl + 128 * (c + 1)],
                            identb,
                        )
                xq_t[u] = None
                xqt = xqtp.tile([P, 512 * nb], BF16, tag="xqt")
                nc.scalar.copy(xqt[:], xqt_ps[:])
                y_ps = y_ps_pool.tile([P, 512 * nb], F32, tag="y")
                for bl in range(nb):
                    for c in range(4):
                        nc.tensor.matmul(
                            y_ps[:, 512 * bl : 512 * (bl + 1)],
                            xqt[:, 512 * bl + 128 * c : 512 * bl + 128 * (c + 1)],
                            wdq[c],
                            start=(c == 0),
                            stop=(c == 3),
                        )
                return y_ps

            def emit_store(u, y_ps):
                r0, nb = units[u]
                y_sb = ysbp.tile([P, 512 * nb], F32, tag="ysb")
                nc.vector.tensor_tensor(
                    out=y_sb[:], in0=y_ps[:], in1=biasb[:, 0 : 512 * nb], op=AL.add
                )
                # stores ride the otherwise-idle Pool engine's SWDGE path: a
                # store whose data is not ready then never head-blocks the SP
                # sequencer that dispatches the x loads.
                nc.gpsimd.dma_start(
                    out=ys[r0 : r0 + 128 * nb, :].rearrange("(b p) f -> p b f", p=P),
                    in_=y_sb[:].rearrange("p (b f) -> p b f", b=nb),
                )

            # prologue: first x load leads the SP queue so its transfer
            # hides the wq8 dispatch latency; wq8 rides second
            emit_load(0)
            if n_units > 1:
                emit_load(1)
            emit_quant(0)
            emit_deq_dve()
            for u in range(n_units):
                if u + 2 < n_units:
                    emit_load(u + 2)
                y = emit_mm(u)
                if u + 1 < n_units:
                    emit_quant(u + 1)
                emit_store(u, y)

    if split_waits:
        _split_waits(nc)
    return nc


# ---------------------------------------------------------------------------
def _prep_weights(w, b):
    """Exact reference weight fake-quant (tiny [512,512], f32 to match the
    reference bit-for-bit), with the joint dequant scale folded in."""
    w = np.ascontiguousarray(w, dtype=np.float32)
    w_bound = np.max(np.abs(w), axis=0, keepdims=True)                  # [1, F]
    w_scale = (np.float32(CLIP) / np.maximum(w_bound, np.float32(EPS))).astype(
        np.float32
    )
    w_q = np.clip(np.rint(w * w_scale), -CLIP, CLIP).astype(np.float32)  # ints
    inv = (np.float32(1.0) / (np.float32(A_SCALE) * w_scale)).astype(np.float32)
    # ship exact int8 w_q (half the DMA bytes of bf16); the device multiplies
    # by inv and rounds to bf16 itself
    wq8 = np.ascontiguousarray(
        w_q.astype(np.int8).reshape(4, P, F_DIM).transpose(1, 0, 2).reshape(P, 2048)
    )
    inv_hi = inv.astype(ml_dtypes.bfloat16)
    inv_lo = (inv - inv_hi.astype(np.float32)).astype(ml_dtypes.bfloat16)
    rows3 = np.zeros((1, 3 * F_DIM), dtype=ml_dtypes.bfloat16)
    rows3[0, :F_DIM] = np.asarray(b, dtype=np.float32).astype(ml_dtypes.bfloat16)
    rows3[0, F_DIM : 2 * F_DIM] = inv_hi.reshape(F_DIM)
    rows3[0, 2 * F_DIM :] = inv_lo.reshape(F_DIM)
    return wq8, rows3


_NC_CACHE = None


def kernel(x, kernel, bias):
    global _NC_CACHE
    if _NC_CACHE is None:
        _NC_CACHE = build_bass()
    nc = _NC_CACHE

    x = np.ascontiguousarray(x, dtype=np.float32)
    wq8, rows3 = _prep_weights(kernel, bias)

    in_maps = []
    for i in range(N_CORES):
        in_maps.append(
            {
                "xs": x[i * ROWS_PER_CORE : (i + 1) * ROWS_PER_CORE],
                "wq8": wq8,
                "rows3": rows3,
            }
        )
    res = run_bass_kernel_spmd(nc, in_maps, core_ids=list(range(N_CORES)))
    return np.concatenate([res.results[i]["ys"] for i in range(N_CORES)], axis=0)
